# revision 1
# baseline (speedup 1.0000x reference)
"""Contrastive SSL loss on 8 Trainium2 NeuronCores.

Strategy (data parallel over batch dim N=8, one sequence per core):
  - Each core holds the full feature table z (replicated, 4MB) and its own
    sequence's context c[i] plus negative indices.
  - On device: normalize z columns and c columns, compute the full similarity
    matrix S = cn^T @ zn (512 x 4096, bf16 matmul, fp32 accum), evacuate PSUM
    through the ACT engine as E = exp(S / TEMP), gather the 101 needed scalars
    per position (1 positive + 100 negatives) with the GPSIMD ap_gather
    instruction, masked row-reduce to sum(exp), take log, and subtract the
    positive logit term (computed separately from the device's own z slice so
    the program is identical across cores).
  - Each core emits one fp32 partial: sum_l [ log(sumexp_l) - pos_l/TEMP ].
    Host sums the 8 partials and divides by N*L.
"""

import numpy as np

N, C, L, K = 8, 256, 512, 100
NL = N * L            # 4096 flat positions
KP = K + 1            # gathered values per position (pos + negs)
GRP = 16 * KP         # gather list length per 16-partition group = 1616
NT = L // 128         # 4 partition tiles of l per core
TEMP = 0.5

_CACHE = {}


def _build_nc():
    import concourse.bacc as bacc
    import concourse.tile as tile
    import concourse.mybir as mybir

    f32 = mybir.dt.float32
    bf16 = mybir.dt.bfloat16
    i16 = mybir.dt.int16

    nc = bacc.Bacc("TRN2", target_bir_lowering=False)
    z_in = nc.dram_tensor("z_in", [N, C, L], f32, kind="ExternalInput")
    c_in = nc.dram_tensor("c_in", [C, L], f32, kind="ExternalInput")
    zs_in = nc.dram_tensor("zs_in", [C, L], f32, kind="ExternalInput")
    idx_in = nc.dram_tensor("idx_in", [NT, 128, KP], i16, kind="ExternalInput")
    mask_in = nc.dram_tensor("mask_in", [128, 16], f32, kind="ExternalInput")
    loss_out = nc.dram_tensor("loss_out", [1, 1], f32, kind="ExternalOutput")

    with tile.TileContext(nc) as tc:
        with (
            tc.tile_pool(name="big", bufs=1) as big,
            tc.tile_pool(name="work", bufs=2) as work,
            tc.tile_pool(name="small", bufs=1) as small,
            tc.tile_pool(name="psum", bufs=2, space="PSUM") as psum,
        ):
            # ---- load inputs (z/c/zsel cast fp32->bf16 during DMA) ----
            zbf = []
            for kc in range(2):
                t = big.tile([128, NL], bf16, tag=f"zbf{kc}")
                src = z_in[:, kc * 128:(kc + 1) * 128, :].rearrange("n p l -> p n l")
                nc.gpsimd.dma_start(
                    out=t[:].rearrange("p (n l) -> p n l", n=N), in_=src
                )
                zbf.append(t)
            cbf = []
            zsbf = []
            for kc in range(2):
                t = small.tile([128, L], bf16, tag=f"cbf{kc}")
                nc.gpsimd.dma_start(out=t, in_=c_in[kc * 128:(kc + 1) * 128, :])
                cbf.append(t)
                t2 = small.tile([128, L], bf16, tag=f"zsbf{kc}")
                nc.gpsimd.dma_start(out=t2, in_=zs_in[kc * 128:(kc + 1) * 128, :])
                zsbf.append(t2)
            idx_sb = small.tile([128, NT * KP], i16, tag="idx")
            nc.sync.dma_start(
                out=idx_sb[:].rearrange("p (t w) -> p t w", t=NT),
                in_=idx_in.rearrange("t p w -> p t w"),
            )
            mask_sb = small.tile([128, 16], f32, tag="mask")
            nc.sync.dma_start(out=mask_sb, in_=mask_in[:, :])
            ones_sb = small.tile([128, 128], bf16, tag="ones")
            nc.vector.memset(ones_sb, 1.0)

            # ---- column norms of z (replicated across partitions via M=128
            #      all-ones stationary operand), then rinv and zn ----
            zsq = []
            for kc in range(2):
                t = big.tile([128, NL], bf16, tag=f"zsq{kc}")
                nc.vector.tensor_mul(t, zbf[kc], zbf[kc])
                zsq.append(t)
            norm_z = big.tile([128, NL], f32, tag="norm_z")
            for h in range(2):
                ps = psum.tile([128, 2048], f32, tag="ps")
                for kc in range(2):
                    for nt in range(4):
                        nc.tensor.matmul(
                            ps[:, nt * 512:(nt + 1) * 512],
                            ones_sb,
                            zsq[kc][:, h * 2048 + nt * 512: h * 2048 + (nt + 1) * 512],
                            start=(kc == 0),
                            stop=(kc == 1),
                        )
                nc.scalar.sqrt(norm_z[:, h * 2048:(h + 1) * 2048], ps)
            rinv_z = big.tile([128, NL], f32, tag="rinv_z")
            nc.vector.reciprocal_approx_fast(rinv_z, norm_z)
            zn = []
            for kc in range(2):
                t = big.tile([128, NL], bf16, tag=f"zn{kc}")
                nc.vector.tensor_mul(t, zbf[kc], rinv_z)
                zn.append(t)

            # ---- column norms of c, cn ----
            csq = []
            for kc in range(2):
                t = small.tile([128, L], bf16, tag=f"csq{kc}")
                nc.vector.tensor_mul(t, cbf[kc], cbf[kc])
                csq.append(t)
            ps_c = psum.tile([128, 512], f32, tag="ps")
            for kc in range(2):
                nc.tensor.matmul(ps_c, ones_sb, csq[kc], start=(kc == 0), stop=(kc == 1))
            norm_c = small.tile([128, L], f32, tag="norm_c")
            nc.scalar.sqrt(norm_c, ps_c)
            rinv_c = small.tile([128, L], f32, tag="rinv_c")
            nc.vector.reciprocal_approx_fast(rinv_c, norm_c)
            cn = []
            for kc in range(2):
                t = small.tile([128, L], bf16, tag=f"cn{kc}")
                nc.vector.tensor_mul(t, cbf[kc], rinv_c)
                cn.append(t)

            # ---- positive-logit path from the core's own z slice ----
            ps_zs = psum.tile([128, 512], f32, tag="ps")
            for kc in range(2):
                zssq = small.tile([128, L], bf16, tag=f"zssq{kc}")
                nc.vector.tensor_mul(zssq, zsbf[kc], zsbf[kc])
                nc.tensor.matmul(ps_zs, ones_sb, zssq, start=(kc == 0), stop=(kc == 1))
            norm_zs = small.tile([128, L], f32, tag="norm_zs")
            nc.scalar.sqrt(norm_zs, ps_zs)
            rinv_zs = small.tile([128, L], f32, tag="rinv_zs")
            nc.vector.reciprocal_approx_fast(rinv_zs, norm_zs)
            ps_pos = psum.tile([128, 512], f32, tag="ps")
            for kc in range(2):
                pd = small.tile([128, L], bf16, tag=f"pd{kc}")
                nc.vector.tensor_mul(pd, cbf[kc], zsbf[kc])
                nc.tensor.matmul(ps_pos, ones_sb, pd, start=(kc == 0), stop=(kc == 1))
            raw_dot = small.tile([128, L], f32, tag="raw_dot")
            nc.scalar.copy(raw_dot, ps_pos)
            cos_pos = small.tile([128, L], f32, tag="cos_pos")
            nc.vector.tensor_mul(cos_pos, raw_dot, rinv_c)
            nc.vector.tensor_mul(cos_pos, cos_pos, rinv_zs)
            pos_sum = small.tile([128, 1], f32, tag="pos_sum")
            nc.vector.reduce_sum(pos_sum, cos_pos, axis=mybir.AxisListType.X)

            # ---- main loop over the 4 l-tiles ----
            se_all = small.tile([128, NT], f32, tag="se_all")
            for t in range(NT):
                e_t = work.tile([128, NL], f32, tag="E")
                for h in range(2):
                    ps = psum.tile([128, 2048], f32, tag="ps")
                    for kc in range(2):
                        lhs = cn[kc][:, t * 128:(t + 1) * 128]
                        for nt in range(4):
                            j0 = h * 2048 + nt * 512
                            nc.tensor.matmul(
                                ps[:, nt * 512:(nt + 1) * 512],
                                lhs,
                                zn[kc][:, j0:j0 + 512],
                                start=(kc == 0),
                                stop=(kc == 1),
                            )
                    nc.scalar.activation(
                        out=e_t[:, h * 2048:(h + 1) * 2048],
                        in_=ps,
                        func=mybir.ActivationFunctionType.Exp,
                        scale=1.0 / TEMP,
                    )
                eg = work.tile([128, GRP], f32, tag="EG")
                nc.gpsimd.ap_gather(
                    out_ap=eg,
                    in_ap=e_t,
                    idxs_ap=idx_sb[:, t * KP:(t + 1) * KP],
                    channels=128,
                    num_elems=NL,
                    d=1,
                    num_idxs=GRP,
                )
                se16 = work.tile([128, 16], f32, tag="se16")
                nc.vector.reduce_sum(
                    se16, eg[:].rearrange("p (j k) -> p j k", j=16),
                    axis=mybir.AxisListType.X,
                )
                masked = work.tile([128, 16], f32, tag="masked")
                nc.vector.tensor_mul(masked, se16, mask_sb)
                nc.vector.reduce_sum(
                    se_all[:, t:t + 1], masked, axis=mybir.AxisListType.X
                )

            # ---- finalize: sum_l ln(sumexp) - (1/TEMP) * sum_l cos_pos ----
            ln_all = small.tile([128, NT], f32, tag="ln_all")
            nc.scalar.activation(ln_all, se_all, mybir.ActivationFunctionType.Ln)
            combo = small.tile([128, 2], f32, tag="combo")
            nc.vector.reduce_sum(combo[:, 0:1], ln_all, axis=mybir.AxisListType.X)
            nc.scalar.mul(combo[:, 1:2], pos_sum, 1.0 / TEMP)
            ones1 = small.tile([128, 1], f32, tag="ones1")
            nc.vector.memset(ones1, 1.0)
            # partition-sum of [lse_sum, scaled_pos_sum] via a 1-column matmul
            ps_fin = psum.tile([128, 2], f32, tag="ps")
            nc.tensor.matmul(ps_fin[0:1, :], ones1, combo, start=True, stop=True)
            red = small.tile([128, 2], f32, tag="red")
            nc.scalar.copy(red[0:1, :], ps_fin[0:1, :])
            final = small.tile([128, 1], f32, tag="final")
            # pos term was summed over l already and is replicated across
            # partitions, so the matmul multiplied it by 128 - undo that.
            nc.scalar.mul(final[0:1, :], red[0:1, 1:2], -1.0 / 128.0)
            nc.vector.tensor_add(final[0:1, :], final[0:1, :], red[0:1, 0:1])
            nc.sync.dma_start(out=loss_out[:, :], in_=final[0:1, :])

    nc.compile()
    return nc


def _prepare_core_inputs(z, c, neg_inds):
    """Host-side sharding + gather-index packing. Returns list of in_maps."""
    z = np.ascontiguousarray(np.asarray(z, dtype=np.float32))
    c = np.asarray(c, dtype=np.float32)
    idx = np.asarray(neg_inds).astype(np.int64)          # (N, L, K) in [0, NL)

    # per-core gather lists: [pos, neg0..neg99] per l
    pos = (np.arange(N)[:, None] * L + np.arange(L)[None, :])  # (N, L)
    lists = np.concatenate([pos[:, :, None], idx], axis=2)     # (N, L, KP)

    # per-partition window-select mask, shared across cores
    p = np.arange(128)[:, None]
    j = np.arange(16)[None, :]
    maskw = (j == (p % 16)).astype(np.float32)                 # (128, 16)

    rows = np.arange(GRP) % 16
    cols = np.arange(GRP) // 16

    in_maps = []
    for i in range(N):
        a = lists[i].reshape(NT, 8, GRP)                       # (t, g, m)
        idxw = np.zeros((NT, 128, KP), dtype=np.int16)
        for g in range(8):
            idxw[:, 16 * g + rows, cols] = a[:, g, :]
        in_maps.append({
            "z_in": z,
            "c_in": np.ascontiguousarray(c[i, :, 1:]),
            "zs_in": np.ascontiguousarray(z[i]),
            "idx_in": idxw,
            "mask_in": maskw,
        })
    return in_maps


def kernel(z, c, neg_inds):
    from concourse.bass_utils import run_bass_kernel_spmd

    if "nc" not in _CACHE:
        _CACHE["nc"] = _build_nc()
    nc = _CACHE["nc"]

    in_maps = _prepare_core_inputs(z, c, neg_inds)
    res = run_bass_kernel_spmd(nc, in_maps, core_ids=list(range(N)))
    partials = [r["loss_out"][0, 0] for r in res.results]
    loss = np.float32(np.sum(np.asarray(partials, dtype=np.float64)) / (N * L))
    return np.asarray(loss, dtype=np.float32)



# revision 4
# speedup vs baseline: 3.7220x; 3.7220x over previous
"""Contrastive SSL loss on 8 Trainium2 NeuronCores.

Strategy (data parallel over batch dim N=8, one sequence per core):
  Each core holds the full normalized feature table zn (bf16, replicated)
  and its own sequence's normalized context cn.  The row sums
  sum_k exp(logit_k / T) over the 101 sampled logits per position are
  computed WITHOUT any gather:

    sum_k exp(s[l, j_k] / T)  ==  sum_j cnt[l, j] * exp(s[l, j] / T)
                              ==  sum_j exp((s[l, j] + T*ln cnt[l, j]) / T)

  where cnt[l, j] counts how often column j is sampled for row l (the
  positive column i*L+l counts once too; cnt=0 cells get B = -15 so they
  vanish).  B = T*ln(cnt) is precomputed on the host, pre-loaded into
  PSUM via an identity-matrix matmul, and the similarity matmul
  accumulates on top (start=False).  A single scalar-engine Exp pass
  with accum_out then produces the weighted row sums for free - the
  slow GPSIMD ap_gather of the old design is gone entirely.

  Each core outputs sum_l ln(sumexp_l).  The host adds the (tiny)
  positive-logit term and averages.
"""

import numpy as np

N, C, L, K = 8, 256, 512, 100
NL = N * L            # 4096 flat z columns
NT = L // 128         # 4 l-tiles per core
TEMP = 0.5
NEG_FILL = -15.0      # B value for cnt=0 -> exp adds ~e-26 per cell, negligible

_CACHE = {}


def _build_nc():
    import concourse.bacc as bacc
    import concourse.tile as tile
    import concourse.mybir as mybir

    f32 = mybir.dt.float32
    bf16 = mybir.dt.bfloat16

    nc = bacc.Bacc("TRN2", target_bir_lowering=False)
    z_in = nc.dram_tensor("z_in", [2, 128, NL], bf16, kind="ExternalInput")
    c_in = nc.dram_tensor("c_in", [2, 128, L], bf16, kind="ExternalInput")
    b_in = nc.dram_tensor("b_in", [NT, 128, NL], bf16, kind="ExternalInput")
    eye_in = nc.dram_tensor("eye_in", [128, 128], bf16, kind="ExternalInput")
    loss_out = nc.dram_tensor("loss_out", [1, 1], f32, kind="ExternalOutput")

    EXP = mybir.ActivationFunctionType.Exp
    LN = mybir.ActivationFunctionType.Ln
    SQRT = mybir.ActivationFunctionType.Sqrt
    COPY = mybir.ActivationFunctionType.Copy

    with tile.TileContext(nc) as tc:
        with (
            tc.tile_pool(name="big", bufs=1) as big,
            tc.tile_pool(name="bstream", bufs=3) as bstream,
            tc.tile_pool(name="work", bufs=2) as work,
            tc.tile_pool(name="small", bufs=1) as small,
            tc.tile_pool(name="psum", bufs=2, space="PSUM") as psum,
        ):
            # ---- input DMAs (z on gpsimd queue, rest on sync) ----
            zbf = []
            for kc in range(2):
                t = big.tile([128, NL], bf16, tag=f"zbf{kc}")
                nc.gpsimd.dma_start(out=t, in_=z_in[kc])
                zbf.append(t)
            cbf = []
            for kc in range(2):
                t = small.tile([128, L], bf16, tag=f"cbf{kc}")
                nc.sync.dma_start(out=t, in_=c_in[kc])
                cbf.append(t)
            eye = small.tile([128, 128], bf16, tag="eye")
            nc.sync.dma_start(out=eye, in_=eye_in[:, :])
            ones_sb = small.tile([128, 128], bf16, tag="ones")
            nc.vector.memset(ones_sb, 1.0)

            # ---- c column norms -> cn (bf16) ----
            csq = []
            for kc in range(2):
                t = small.tile([128, L], bf16, tag=f"csq{kc}")
                nc.vector.tensor_mul(t, cbf[kc], cbf[kc])
                csq.append(t)
            ps_c = psum.tile([128, L], f32, tag="ps")
            for kc in range(2):
                nc.tensor.matmul(ps_c, ones_sb, csq[kc], start=(kc == 0), stop=(kc == 1))
            r2_c = small.tile([128, L], f32, tag="r2c")
            nc.vector.reciprocal_approx_fast(r2_c, ps_c)
            rinv_c = small.tile([128, L], bf16, tag="rinvc")
            nc.scalar.activation(rinv_c, r2_c, SQRT)
            cn = []
            for kc in range(2):
                t = small.tile([128, L], bf16, tag=f"cn{kc}")
                nc.vector.tensor_mul(t, cbf[kc], rinv_c)
                cn.append(t)

            # ---- z column norms -> zn (bf16) ----
            zsq = []
            for kc in range(2):
                t = big.tile([128, NL], bf16, tag=f"zsq{kc}")
                nc.vector.tensor_mul(t, zbf[kc], zbf[kc])
                zsq.append(t)
            r2_z = big.tile([128, NL], f32, tag="r2z")
            for h in range(2):
                ps = psum.tile([128, 2048], f32, tag="ps")
                for kc in range(2):
                    for q in range(4):
                        nc.tensor.matmul(
                            ps[:, q * 512:(q + 1) * 512],
                            ones_sb,
                            zsq[kc][:, h * 2048 + q * 512: h * 2048 + (q + 1) * 512],
                            start=(kc == 0),
                            stop=(kc == 1),
                        )
                nc.vector.reciprocal_approx_fast(r2_z[:, h * 2048:(h + 1) * 2048], ps)
            rinv_z = big.tile([128, NL], bf16, tag="rinvz")
            for h in range(2):
                nc.scalar.activation(
                    rinv_z[:, h * 2048:(h + 1) * 2048],
                    r2_z[:, h * 2048:(h + 1) * 2048],
                    SQRT,
                )
            zn = []
            for kc in range(2):
                t = big.tile([128, NL], bf16, tag=f"zn{kc}")
                nc.vector.tensor_mul(t, zbf[kc], rinv_z)
                zn.append(t)

            # ---- main loop: 4 l-tiles x 2 halves of 2048 columns ----
            # PSUM <- B (identity matmul), += cn^T zn, then one Exp pass
            # with accum_out gives sum_j cnt*exp(s/T) per row.
            se8 = small.tile([128, NT * 2], f32, tag="se8")
            for lt in range(NT):
                for h in range(2):
                    b_sb = bstream.tile([128, 2048], bf16, tag="b")
                    nc.sync.dma_start(
                        out=b_sb, in_=b_in[lt, :, h * 2048:(h + 1) * 2048]
                    )
                    ps = psum.tile([128, 2048], f32, tag="ps")
                    for q in range(4):
                        nc.tensor.matmul(
                            ps[:, q * 512:(q + 1) * 512],
                            eye,
                            b_sb[:, q * 512:(q + 1) * 512],
                            start=True,
                            stop=False,
                        )
                    for kc in range(2):
                        lhsT = cn[kc][:, lt * 128:(lt + 1) * 128]
                        for q in range(4):
                            j0 = h * 2048 + q * 512
                            nc.tensor.matmul(
                                ps[:, q * 512:(q + 1) * 512],
                                lhsT,
                                zn[kc][:, j0:j0 + 512],
                                start=False,
                                stop=(kc == 1),
                            )
                    scr = work.tile([128, 2048], bf16, tag="scr")
                    nc.scalar.activation(
                        scr, ps, EXP, scale=1.0 / TEMP,
                        accum_out=se8[:, lt * 2 + h: lt * 2 + h + 1],
                    )

            # ---- finalize: sum_l ln(se_l) ----
            se4 = small.tile([128, NT], f32, tag="se4")
            nc.vector.tensor_add(
                se4,
                se8[:].rearrange("p (t c) -> p t c", c=2)[:, :, 0],
                se8[:].rearrange("p (t c) -> p t c", c=2)[:, :, 1],
            )
            ln4 = small.tile([128, NT], f32, tag="ln4")
            lse1 = small.tile([128, 1], f32, tag="lse1")
            nc.scalar.activation(ln4, se4, LN, accum_out=lse1)
            ones1 = small.tile([128, 1], f32, tag="ones1")
            nc.vector.memset(ones1, 1.0)
            ps_f = psum.tile([128, 1], f32, tag="ps")
            nc.tensor.matmul(ps_f[0:1, :], ones1, lse1, start=True, stop=True)
            final = small.tile([128, 1], f32, tag="final")
            nc.scalar.copy(final[0:1, :], ps_f[0:1, :])
            nc.sync.dma_start(out=loss_out[:, :], in_=final[0:1, :])

    nc.compile()
    return nc


def _prepare_core_inputs(z, c, neg_inds):
    """Host-side prep: bf16 casts, count matrix -> B = T*ln(cnt), pos term."""
    import ml_dtypes

    bf16 = ml_dtypes.bfloat16
    z = np.asarray(z, dtype=np.float32)
    c = np.asarray(c, dtype=np.float32)
    idx = np.asarray(neg_inds).astype(np.int64)            # (N, L, K) in [0, NL)

    # shared normalized-input-free tensors
    z_flat = np.transpose(z, (1, 0, 2)).reshape(C, NL)     # (C, NL)
    z_bf = np.ascontiguousarray(z_flat.reshape(2, 128, NL)).astype(bf16)

    eye = np.eye(128, dtype=np.float32).astype(bf16)

    # host-side positive-logit sums (0.025% of the FLOPs; rest on device)
    eps = 1e-8
    c_seq = np.transpose(c[:, :, 1:], (0, 2, 1))           # (N, L, C)
    z_seq = np.transpose(z, (0, 2, 1))                     # (N, L, C)
    cn = c_seq / np.maximum(np.linalg.norm(c_seq, axis=-1, keepdims=True), eps)
    zn = z_seq / np.maximum(np.linalg.norm(z_seq, axis=-1, keepdims=True), eps)
    pos_sum = float(np.einsum("nlc,nlc->", cn, zn, dtype=np.float64))

    in_maps = []
    for i in range(N):
        cnt = np.zeros((L, NL), dtype=np.float32)
        flat = (np.arange(L)[:, None] * NL + idx[i]).ravel()
        np.add.at(cnt.reshape(-1), flat, 1.0)
        cnt[np.arange(L), i * L + np.arange(L)] += 1.0     # positive column
        B = np.full((L, NL), NEG_FILL, dtype=np.float32)
        nz = cnt > 0
        B[nz] = TEMP * np.log(cnt[nz])
        in_maps.append({
            "z_in": z_bf,
            "c_in": np.ascontiguousarray(
                c[i, :, 1:].reshape(2, 128, L)).astype(bf16),
            "b_in": np.ascontiguousarray(B.reshape(NT, 128, NL)).astype(bf16),
            "eye_in": eye,
        })
    return in_maps, pos_sum


def kernel(z, c, neg_inds):
    from concourse.bass_utils import run_bass_kernel_spmd

    if "nc" not in _CACHE:
        _CACHE["nc"] = _build_nc()
    nc = _CACHE["nc"]

    in_maps, pos_sum = _prepare_core_inputs(z, c, neg_inds)
    res = run_bass_kernel_spmd(nc, in_maps, core_ids=list(range(N)))
    lse_sum = np.sum(
        np.asarray([r["loss_out"][0, 0] for r in res.results], dtype=np.float64)
    )
    loss = (lse_sum - pos_sum / TEMP) / (N * L)
    return np.asarray(loss, dtype=np.float32)
